# revision 8
# baseline (speedup 1.0000x reference)
"""Distributed single-head causal attention for Trainium2 (8 NeuronCores).

Problem: x:[4,2048,1024] f32, Wq/Wk/Wv/Wo:[1024,1024], b*:[1024]
  q = x@Wq.T+bq; k = x@Wk.T+bk; v = x@Wv.T+bv
  scores = (q@k.T)/sqrt(1024) causal-masked; out = softmax(scores)@v @Wo.T + bo

Sharding (data-parallel + pair K/V exchange, causal-balanced, strict-SPMD):
  8 cores = 4 batches x 2 cores/batch. The 16 query blocks (128 rows) of a
  batch are split by parity: group rank 0 (even core) takes odd blocks, rank
  1 takes even blocks. Every core runs 8 "slots" with the same compile-time
  key-range schedule T_s = 256*(s+1) -> identical SPMD instruction stream and
  perfectly balanced causal work. Each core projects K/V only for its OWN
  1024 rows; the pair exchanges halves via AllGather on the CC engine while
  the PE computes the other projections (full overlap). Host gathers query
  rows per core and builds the per-core additive diagonal mask.

Compute (bf16 matmuls, fp32 PSUM), per core:
  KTh[e,own-s] = WkT.T @ xqT (+bk)     -> AllGather pair -> KT[e, 2048]
  Vh[own-t,d]  = xqT-tiles.T @ WvT (+bv) -> AllGather pair -> V[2048, d]
  QT[e,s]      = WqT.T @ xqT (x1/32, +bq/32)
  per slot: scores = QT.T @ KT (psum, ec-outer); +mask; softmax
  (reduce_max negate -> Exp bias=-max accum_out=l); attnT via one batched
  bf16 xbar DMA-transpose; ctxT[d,s] = V.T-tiles @ attnT (ragged suffix);
  out[s,e] = ctxT.T @ WoT * (1/l) + bo (fused DVE scalar_tensor_tensor).
"""

import sys

if "/opt/trn_rl_repo" not in sys.path:
    sys.path.insert(0, "/opt/trn_rl_repo")

import numpy as np
import ml_dtypes

import concourse.bass as bass
import concourse.mybir as mybir
from concourse import bacc
from concourse.bass_utils import run_bass_kernel_spmd
from concourse.tile import TileContext

B, S, D = 4, 2048, 1024
NB = S // 128          # 16 key blocks per batch
NSLOT = 8              # query slots per core
EC = D // 128          # 8 feature chunks
F32 = mybir.dt.float32
BF16 = mybir.dt.bfloat16
NEG = -1.0e9
GROUPS = [[0, 1], [2, 3], [4, 5], [6, 7]]

_compiled = None


def _slot_T(s):
    return 256 * (s + 1)


def _build():
    nc = bacc.Bacc("TRN2", target_bir_lowering=False, debug=False, num_devices=8)

    xqT = nc.dram_tensor("xqT", [128, EC, 1024], BF16, kind="ExternalInput")
    wqT = nc.dram_tensor("wqT", [128, EC, D], BF16, kind="ExternalInput")
    wkT = nc.dram_tensor("wkT", [128, EC, D], BF16, kind="ExternalInput")
    wvT = nc.dram_tensor("wvT", [128, EC, D], BF16, kind="ExternalInput")
    woT = nc.dram_tensor("woT", [128, EC, D], BF16, kind="ExternalInput")
    bq_d = nc.dram_tensor("bq", [128, EC], F32, kind="ExternalInput")
    bk_d = nc.dram_tensor("bk", [128, EC], F32, kind="ExternalInput")
    bv_d = nc.dram_tensor("bv", [1, D], F32, kind="ExternalInput")
    bo_d = nc.dram_tensor("bo", [1, D], F32, kind="ExternalInput")
    mask_d = nc.dram_tensor("mask", [128, NSLOT, 256], F32, kind="ExternalInput")
    out_d = nc.dram_tensor("out", [NSLOT * 128, D], F32, kind="ExternalOutput")

    inv = 1.0 / 32.0

    with TileContext(nc) as tc:
        with (
            tc.tile_pool(name="persist", bufs=1) as persist,
            tc.tile_pool(name="small", bufs=1) as small,
            tc.tile_pool(name="dram", bufs=1, space="DRAM") as dram,
        ):
            QT = persist.tile([128, EC, 1024], BF16, tag="QT")
            KT = persist.tile([128, EC, 2, 1024], BF16, tag="KT")
            V = persist.tile([128, NB, D], BF16, tag="V")
            MASK = small.tile([128, NSLOT, 256], F32, tag="MASK")
            BQ = small.tile([128, EC], F32, tag="BQ")
            BK = small.tile([128, EC], F32, tag="BK")
            RL = small.tile([128, NSLOT], F32, tag="RL")
            BOF = small.tile([128, D], F32, tag="BOF")

            khalfA = dram.tile([128, EC, 512], BF16, tag="khalfA")
            khalfB = dram.tile([128, EC, 512], BF16, tag="khalfB")
            kgathA = dram.tile([2, 128, EC, 512], BF16, tag="kgathA")
            kgathB = dram.tile([2, 128, EC, 512], BF16, tag="kgathB")
            vhalfA = dram.tile([128, 4, D], BF16, tag="vhalfA")
            vhalfB = dram.tile([128, 4, D], BF16, tag="vhalfB")
            vgathA = dram.tile([2, 128, 4, D], BF16, tag="vgathA")
            vgathB = dram.tile([2, 128, 4, D], BF16, tag="vgathB")

            # ---- phase A: projections + pair K/V exchange ----
            with (
                tc.tile_pool(name="xin", bufs=1) as xin,
                tc.tile_pool(name="wts", bufs=1) as wts,
                tc.tile_pool(name="pa_psum", bufs=6, space="PSUM") as pa_psum,
            ):
                XQ = xin.tile([128, EC, 1024], BF16, tag="XQ")
                KTh = xin.tile([128, EC, 1024], BF16, tag="KTh")
                Vh = xin.tile([128, NSLOT, D], BF16, tag="Vh")
                WQ = wts.tile([128, EC, D], BF16, tag="WQ")
                WK = wts.tile([128, EC, D], BF16, tag="WK")
                WV = wts.tile([128, EC, D], BF16, tag="WV")
                BVF = xin.tile([128, D], F32, tag="BVF")

                # loads in compute-need order: K-half first
                for dc in range(EC):
                    nc.sync.dma_start(out=XQ[:, dc, :], in_=xqT[:, dc, :])
                    nc.sync.dma_start(out=WK[:, dc, :], in_=wkT[:, dc, :])
                nc.sync.dma_start(out=BK[:, :], in_=bk_d[:, :])
                for dc in range(EC):
                    nc.sync.dma_start(out=WV[:, dc, :], in_=wvT[:, dc, :])
                bv_row = small.tile([1, D], F32, tag="bv_row")
                nc.sync.dma_start(out=bv_row[:, :], in_=bv_d[:, :])
                nc.gpsimd.partition_broadcast(BVF[:, :], bv_row[:1, :])
                for dc in range(EC):
                    nc.sync.dma_start(out=WQ[:, dc, :], in_=wqT[:, dc, :])
                bq_raw = small.tile([128, EC], F32, tag="bq_raw")
                nc.sync.dma_start(out=bq_raw[:, :], in_=bq_d[:, :])
                nc.scalar.mul(BQ[:, :], bq_raw[:, :], inv)
                nc.sync.dma_start(out=MASK[:, :, :], in_=mask_d[:, :, :])
                bo_row = small.tile([1, D], F32, tag="bo_row")
                nc.sync.dma_start(out=bo_row[:, :], in_=bo_d[:, :])
                nc.gpsimd.partition_broadcast(BOF[:, :], bo_row[:1, :])

                # K-half: KTh[e, own-s]; sh-separated waves (sh0 first) so
                # the first K exchange fires early; dc outer for streaming
                def k_like_waves(WMAT, DST, bias_ap, scaled):
                    for sh in range(2):
                        for w in range(2):
                            ec0 = 4 * w
                            pss = [
                                pa_psum.tile(
                                    [128, 512], F32, tag="pa", name=f"pa_{sh}_{w}_{i}"
                                )
                                for i in range(4)
                            ]
                            for dc in range(EC):
                                for i in range(4):
                                    nc.tensor.matmul(
                                        pss[i][:, :],
                                        WMAT[:, dc, (ec0 + i) * 128 : (ec0 + i + 1) * 128],
                                        XQ[:, dc, sh * 512 : (sh + 1) * 512],
                                        start=(dc == 0),
                                        stop=(dc == EC - 1),
                                    )
                            for i in range(4):
                                ec = ec0 + i
                                if scaled:
                                    nc.vector.tensor_scalar(
                                        out=DST[:, ec, sh * 512 : (sh + 1) * 512],
                                        in0=pss[i][:, :],
                                        scalar1=inv,
                                        scalar2=bias_ap[:, ec : ec + 1],
                                        op0=mybir.AluOpType.mult,
                                        op1=mybir.AluOpType.add,
                                    )
                                else:
                                    nc.vector.tensor_scalar(
                                        out=DST[:, ec, sh * 512 : (sh + 1) * 512],
                                        in0=pss[i][:, :],
                                        scalar1=bias_ap[:, ec : ec + 1],
                                        scalar2=None,
                                        op0=mybir.AluOpType.add,
                                    )

                k_like_waves(WK, KTh, BK, scaled=False)
                # exchange K in halves; out-DMAs on the scalar HWDGE queue,
                # collectives + load-backs in-order on the gpsimd queue
                nc.scalar.dma_start(out=khalfA[:, :, :], in_=KTh[:, :, 0:512])
                nc.scalar.dma_start(out=khalfB[:, :, :], in_=KTh[:, :, 512:1024])
                nc.gpsimd.collective_compute(
                    "AllGather",
                    mybir.AluOpType.bypass,
                    replica_groups=GROUPS,
                    ins=[khalfA.opt()],
                    outs=[kgathA.opt()],
                )
                nc.gpsimd.collective_compute(
                    "AllGather",
                    mybir.AluOpType.bypass,
                    replica_groups=GROUPS,
                    ins=[khalfB.opt()],
                    outs=[kgathB.opt()],
                )

                # V-half: Vh[own-t, d]; slot-order waves (2 slots x 2 dh)
                Vv = V[:, :, :].rearrange("p (s b) d -> p s b d", b=2)
                for w in range(4):
                    s0 = 2 * w
                    pss = [
                        pa_psum.tile([128, 512], F32, tag="pa", name=f"pav{w}_{i}")
                        for i in range(4)
                    ]
                    for dc in range(EC):
                        for i, (sl, dh) in enumerate(
                            [(s0, 0), (s0, 1), (s0 + 1, 0), (s0 + 1, 1)]
                        ):
                            nc.tensor.matmul(
                                pss[i][:, :],
                                XQ[:, dc, sl * 128 : (sl + 1) * 128],
                                WV[:, dc, dh * 512 : (dh + 1) * 512],
                                start=(dc == 0),
                                stop=(dc == EC - 1),
                            )
                    for i, (sl, dh) in enumerate(
                        [(s0, 0), (s0, 1), (s0 + 1, 0), (s0 + 1, 1)]
                    ):
                        nc.vector.tensor_tensor(
                            out=Vh[:, sl, dh * 512 : (dh + 1) * 512],
                            in0=pss[i][:, :],
                            in1=BVF[:, dh * 512 : (dh + 1) * 512],
                            op=mybir.AluOpType.add,
                        )
                    if w == 1:
                        nc.scalar.dma_start(out=vhalfA[:, :, :], in_=Vh[:, 0:4, :])
                        nc.gpsimd.collective_compute(
                            "AllGather",
                            mybir.AluOpType.bypass,
                            replica_groups=GROUPS,
                            ins=[vhalfA.opt()],
                            outs=[vgathA.opt()],
                        )

                    if w == 3:
                        nc.scalar.dma_start(out=vhalfB[:, :, :], in_=Vh[:, 4:8, :])
                        nc.gpsimd.collective_compute(
                            "AllGather",
                            mybir.AluOpType.bypass,
                            replica_groups=GROUPS,
                            ins=[vhalfB.opt()],
                            outs=[vgathB.opt()],
                        )
                        # all load-backs AFTER every collective trigger:
                        # the CC engine is serial, so triggers must not wait
                        # behind a load-back on the in-order gpsimd queue
                        for r in range(2):
                            nc.gpsimd.dma_start(
                                out=KT[:, :, r, 0:512], in_=kgathA[r, :, :, :]
                            )
                        for r in range(2):
                            nc.gpsimd.dma_start(
                                out=KT[:, :, r, 512:1024], in_=kgathB[r, :, :, :]
                            )
                        for r in range(2):
                            nc.gpsimd.dma_start(
                                out=Vv[:, 0:4, r, :], in_=vgathA[r, :, :, :]
                            )
                        for r in range(2):
                            nc.gpsimd.dma_start(
                                out=Vv[:, 4:8, r, :], in_=vgathB[r, :, :, :]
                            )

                # QT (x 1/32, +bq/32), sh0 first so scores g0 unblock early
                k_like_waves(WQ, QT, BQ, scaled=True)

            # ---- phase B + C: attention + output projection ----
            with (
                tc.tile_pool(name="wo", bufs=1) as wo_pool,
                tc.tile_pool(name="att", bufs=5) as att_pool,
                tc.tile_pool(name="attT", bufs=2) as attT_pool,
                tc.tile_pool(name="ctx", bufs=1) as ctx_pool,
                tc.tile_pool(name="stat", bufs=1) as stat_pool,
                tc.tile_pool(name="sc_psum", bufs=3, space="PSUM") as sc_psum,
                tc.tile_pool(name="mm_psum", bufs=2, space="PSUM") as mm_psum,
                tc.tile_pool(name="outbuf", bufs=2) as out_pool,
            ):
                WO = wo_pool.tile([128, EC, D], BF16, tag="WO")
                for dc in range(EC):
                    nc.sync.dma_start(out=WO[:, dc, :], in_=woT[:, dc, :])
                CTXT = ctx_pool.tile([128, EC, 1024], BF16, tag="CTXT")
                NM = stat_pool.tile([128, 2], F32, tag="NM")
                LSUM = stat_pool.tile([128, 2], F32, tag="LS")
                LTOT = stat_pool.tile([128, 1], F32, tag="LT")

                def out_proj(slot):
                    OUTS = out_pool.tile([128, D], F32, tag="outs")
                    for eh in range(2):
                        ps = mm_psum.tile([128, 512], F32, tag="mm")
                        for dc in range(EC):
                            nc.tensor.matmul(
                                ps[:, :],
                                CTXT[:, dc, slot * 128 : (slot + 1) * 128],
                                WO[:, dc, eh * 512 : (eh + 1) * 512],
                                start=(dc == 0),
                                stop=(dc == EC - 1),
                            )
                        nc.vector.scalar_tensor_tensor(
                            out=OUTS[:, eh * 512 : (eh + 1) * 512],
                            in0=ps[:, :],
                            scalar=RL[:, slot : slot + 1],
                            in1=BOF[:, eh * 512 : (eh + 1) * 512],
                            op0=mybir.AluOpType.mult,
                            op1=mybir.AluOpType.add,
                        )
                    nc.sync.dma_start(
                        out=out_d[slot * 128 : (slot + 1) * 128, :], in_=OUTS[:, :]
                    )

                for g in range(2):
                    ATT_T = attT_pool.tile([128, NB, 512], BF16, tag="attT")
                    for j in range(4):
                        slot = g * 4 + j
                        T = _slot_T(slot)
                        nt = T // 128
                        ATT = att_pool.tile([128, S], BF16, tag="att")

                        nparts = (T + 1023) // 1024
                        parts = []
                        for p in range(nparts):
                            w = min(1024, T - p * 1024)
                            sc = sc_psum.tile([128, 1024], F32, tag="sc")
                            parts.append((sc, w))
                        # ec-outer: one LDWEIGHTS per ec covers the whole row
                        KTview = KT[:, :, :, :].rearrange(
                            "p e r (s c) -> p e s r c", c=128
                        )
                        for ec in range(EC):
                            for p, (sc, w) in enumerate(parts):
                                for c0 in range(0, w, 512):
                                    cw = min(512, w - c0)
                                    a0 = p * 1024 + c0
                                    s0, s1 = a0 // 256, (a0 + cw) // 256
                                    nc.tensor.matmul(
                                        sc[:, c0 : c0 + cw],
                                        QT[:, ec, slot * 128 : (slot + 1) * 128],
                                        KTview[:, ec, s0:s1, :, :],
                                        start=(ec == 0),
                                        stop=(ec == EC - 1),
                                    )
                        lsc, lw = parts[-1]
                        nc.vector.tensor_tensor(
                            out=lsc[:, lw - 256 : lw],
                            in0=lsc[:, lw - 256 : lw],
                            in1=MASK[:, slot, :],
                            op=mybir.AluOpType.add,
                        )
                        for p, (sc, w) in enumerate(parts):
                            nc.vector.reduce_max(
                                out=NM[:, p : p + 1],
                                in_=sc[:, :w],
                                axis=mybir.AxisListType.X,
                                negate=True,
                            )
                        if nparts == 2:
                            nc.vector.tensor_tensor(
                                out=NM[:, 0:1],
                                in0=NM[:, 0:1],
                                in1=NM[:, 1:2],
                                op=mybir.AluOpType.min,
                            )
                        for p, (sc, w) in enumerate(parts):
                            nc.scalar.activation(
                                ATT[:, p * 1024 : p * 1024 + w],
                                sc[:, :w],
                                mybir.ActivationFunctionType.Exp,
                                bias=NM[:, 0:1],
                                scale=1.0,
                                accum_out=LSUM[:, p : p + 1],
                            )
                        if nparts == 2:
                            nc.vector.tensor_tensor(
                                out=LTOT[:, 0:1],
                                in0=LSUM[:, 0:1],
                                in1=LSUM[:, 1:2],
                                op=mybir.AluOpType.add,
                            )
                            nc.vector.reciprocal(RL[:, slot : slot + 1], LTOT[:, 0:1])
                        else:
                            nc.vector.reciprocal(RL[:, slot : slot + 1], LSUM[:, 0:1])

                        nc.sync.dma_start_transpose(
                            ATT_T[:, 0:nt, j * 128 : (j + 1) * 128],
                            ATT[:, 0:T],
                        )

                    ntg = _slot_T(g * 4 + 3) // 128
                    for dc in range(EC):
                        ps = mm_psum.tile([128, 512], F32, tag="mm")
                        for tcn in range(ntg):
                            jmin = 0
                            for jj in range(4):
                                if 256 * (g * 4 + jj + 1) >= 128 * (tcn + 1):
                                    jmin = jj
                                    break
                            scol = jmin * 128
                            nc.tensor.matmul(
                                ps[:, scol:512],
                                V[:, tcn, dc * 128 : (dc + 1) * 128],
                                ATT_T[:, tcn, scol:512],
                                start=(tcn == 0),
                                stop=(tcn == ntg - 1),
                            )
                        nc.vector.tensor_copy(
                            CTXT[:, dc, g * 512 : (g + 1) * 512], ps[:, :]
                        )
                    for j in range(4):
                        out_proj(g * 4 + j)

    nc.compile()
    return nc


def _core_blocks(core):
    parity = 0 if core % 2 == 0 else 1  # even core (group rank 0) -> even blocks
    return [2 * s + parity for s in range(NSLOT)]


def _make_in_maps(x, Wq, bq, Wk, bk, Wv, bv, Wo, bo):
    bf = ml_dtypes.bfloat16

    def wt_layout(W):
        return np.ascontiguousarray(
            W.T.astype(bf).reshape(EC, 128, D).transpose(1, 0, 2)
        )

    wq_l, wk_l, wv_l, wo_l = (wt_layout(W) for W in (Wq, Wk, Wv, Wo))
    bq_l = np.ascontiguousarray(bq.reshape(EC, 128).T.astype(np.float32))
    bk_l = np.ascontiguousarray(bk.reshape(EC, 128).T.astype(np.float32))
    bv_l = np.ascontiguousarray(bv.reshape(1, D).astype(np.float32))
    bo_l = np.ascontiguousarray(bo.reshape(1, D).astype(np.float32))

    in_maps = []
    for core in range(8):
        b = core // 2
        blocks = _core_blocks(core)
        xb = np.asarray(x[b], dtype=np.float32)
        xq = np.concatenate([xb[bl * 128 : (bl + 1) * 128] for bl in blocks], axis=0)
        xqT_l = np.ascontiguousarray(
            xq.T.astype(bf).reshape(EC, 128, 1024).transpose(1, 0, 2)
        )
        mask = np.zeros((128, NSLOT, 256), np.float32)
        r = np.arange(128)[:, None]
        jj = np.arange(256)[None, :]
        for s_i, bl in enumerate(blocks):
            lim = bl * 128 + r
            t_idx = 256 * s_i + jj
            mask[:, s_i, :] = np.where(t_idx <= lim, 0.0, NEG)
        in_maps.append(
            {
                "xqT": xqT_l,
                "wqT": wq_l,
                "wkT": wk_l,
                "wvT": wv_l,
                "woT": wo_l,
                "bq": bq_l,
                "bk": bk_l,
                "bv": bv_l,
                "bo": bo_l,
                "mask": mask,
            }
        )
    return in_maps


def _run(inputs, trace=False):
    global _compiled
    if _compiled is None:
        _compiled = _build()
    nc = _compiled
    in_maps = _make_in_maps(**inputs)
    res = run_bass_kernel_spmd(nc, in_maps, core_ids=list(range(8)), trace=trace)
    out = np.zeros((B, S, D), np.float32)
    for core in range(8):
        b = core // 2
        o = res.results[core]["out"]
        for s_i, bl in enumerate(_core_blocks(core)):
            out[b, bl * 128 : (bl + 1) * 128, :] = o[s_i * 128 : (s_i + 1) * 128, :]
    return out, res


def kernel(**inputs):
    out, _ = _run(inputs, trace=False)
    return out


# revision 9
# speedup vs baseline: 1.0887x; 1.0887x over previous
"""Distributed single-head causal attention for Trainium2 (8 NeuronCores).

Problem: x:[4,2048,1024] f32, Wq/Wk/Wv/Wo:[1024,1024], b*:[1024]
  q = x@Wq.T+bq; k = x@Wk.T+bk; v = x@Wv.T+bv
  scores = (q@k.T)/sqrt(1024) causal-masked; out = softmax(scores)@v @Wo.T + bo

Sharding (data-parallel + pair K/V exchange, causal-balanced, strict-SPMD):
  8 cores = 4 batches x 2 cores/batch. The 16 query blocks (128 rows) of a
  batch are split by parity: group rank 0 (even core) takes odd blocks, rank
  1 takes even blocks. Every core runs 8 "slots" with the same compile-time
  key-range schedule T_s = 256*(s+1) -> identical SPMD instruction stream and
  perfectly balanced causal work. Each core projects K/V only for its OWN
  1024 rows; the pair exchanges halves via AllGather on the CC engine while
  the PE computes the other projections (full overlap). Host gathers query
  rows per core and builds the per-core additive diagonal mask.

Compute (bf16 matmuls, fp32 PSUM), per core:
  KTh[e,own-s] = WkT.T @ xqT (+bk)     -> AllGather pair -> KT[e, 2048]
  Vh[own-t,d]  = xqT-tiles.T @ WvT (+bv) -> AllGather pair -> V[2048, d]
  QT[e,s]      = WqT.T @ xqT (x1/32, +bq/32)
  per slot: scores = QT.T @ KT (psum, ec-outer); +mask; softmax
  (reduce_max negate -> Exp bias=-max accum_out=l); attnT via one batched
  bf16 xbar DMA-transpose; ctxT[d,s] = V.T-tiles @ attnT (ragged suffix);
  out[s,e] = ctxT.T @ WoT * (1/l) + bo (fused DVE scalar_tensor_tensor).
"""

import sys

if "/opt/trn_rl_repo" not in sys.path:
    sys.path.insert(0, "/opt/trn_rl_repo")

import numpy as np
import ml_dtypes

import concourse.bass as bass
import concourse.mybir as mybir
from concourse import bacc
from concourse.bass_utils import run_bass_kernel_spmd
from concourse.tile import TileContext

B, S, D = 4, 2048, 1024
NB = S // 128          # 16 key blocks per batch
NSLOT = 8              # query slots per core
EC = D // 128          # 8 feature chunks
F32 = mybir.dt.float32
BF16 = mybir.dt.bfloat16
NEG = -1.0e9
GROUPS = [[0, 1], [2, 3], [4, 5], [6, 7]]

_compiled = None


def _slot_T(s):
    return 256 * (s + 1)


def _build():
    nc = bacc.Bacc("TRN2", target_bir_lowering=False, debug=False, num_devices=8)

    xqT = nc.dram_tensor("xqT", [128, EC, 1024], BF16, kind="ExternalInput")
    wqT = nc.dram_tensor("wqT", [128, EC, D], BF16, kind="ExternalInput")
    wkT = nc.dram_tensor("wkT", [128, EC, D], BF16, kind="ExternalInput")
    wvT = nc.dram_tensor("wvT", [128, EC, D], BF16, kind="ExternalInput")
    woT = nc.dram_tensor("woT", [128, EC, D], BF16, kind="ExternalInput")
    bq_d = nc.dram_tensor("bq", [128, EC], F32, kind="ExternalInput")
    bk_d = nc.dram_tensor("bk", [128, EC], F32, kind="ExternalInput")
    bv_d = nc.dram_tensor("bv", [1, D], F32, kind="ExternalInput")
    bo_d = nc.dram_tensor("bo", [1, D], F32, kind="ExternalInput")
    mask_d = nc.dram_tensor("mask", [128, NSLOT, 256], F32, kind="ExternalInput")
    out_d = nc.dram_tensor("out", [NSLOT * 128, D], F32, kind="ExternalOutput")

    inv = 1.0 / 32.0

    with TileContext(nc) as tc:
        with (
            tc.tile_pool(name="persist", bufs=1) as persist,
            tc.tile_pool(name="small", bufs=1) as small,
            tc.tile_pool(name="dram", bufs=1, space="DRAM") as dram,
        ):
            QT = persist.tile([128, EC, 1024], BF16, tag="QT")
            KT = persist.tile([128, EC, 2, 1024], BF16, tag="KT")
            V = persist.tile([128, NB, D], BF16, tag="V")
            MASK = small.tile([128, NSLOT, 256], F32, tag="MASK")
            BQ = small.tile([128, EC], F32, tag="BQ")
            BK = small.tile([128, EC], F32, tag="BK")
            RL = small.tile([128, NSLOT], F32, tag="RL")
            BOF = small.tile([128, D], F32, tag="BOF")

            khalfA = dram.tile([128, EC, 512], BF16, tag="khalfA")
            khalfB = dram.tile([128, EC, 512], BF16, tag="khalfB")
            kgathA = dram.tile([2, 128, EC, 512], BF16, tag="kgathA")
            kgathB = dram.tile([2, 128, EC, 512], BF16, tag="kgathB")
            vhalfA = dram.tile([128, 4, D], BF16, tag="vhalfA")
            vhalfB = dram.tile([128, 4, D], BF16, tag="vhalfB")
            vgathA = dram.tile([2, 128, 4, D], BF16, tag="vgathA")
            vgathB = dram.tile([2, 128, 4, D], BF16, tag="vgathB")

            # ---- phase A: projections + pair K/V exchange ----
            with (
                tc.tile_pool(name="xin", bufs=1) as xin,
                tc.tile_pool(name="wts", bufs=1) as wts,
                tc.tile_pool(name="pa_psum", bufs=6, space="PSUM") as pa_psum,
            ):
                XQ = xin.tile([128, EC, 1024], BF16, tag="XQ")
                KTh = xin.tile([128, EC, 1024], BF16, tag="KTh")
                Vh = xin.tile([128, NSLOT, D], BF16, tag="Vh")
                WQ = wts.tile([128, EC, D], BF16, tag="WQ")
                WK = wts.tile([128, EC, D], BF16, tag="WK")
                WV = wts.tile([128, EC, D], BF16, tag="WV")
                BVF = xin.tile([128, D], F32, tag="BVF")

                # loads in compute-need order: K-half first
                for dc in range(EC):
                    nc.sync.dma_start(out=XQ[:, dc, :], in_=xqT[:, dc, :])
                    nc.sync.dma_start(out=WK[:, dc, :], in_=wkT[:, dc, :])
                nc.sync.dma_start(out=BK[:, :], in_=bk_d[:, :])
                for dc in range(EC):
                    nc.sync.dma_start(out=WV[:, dc, :], in_=wvT[:, dc, :])
                bv_row = small.tile([1, D], F32, tag="bv_row")
                nc.sync.dma_start(out=bv_row[:, :], in_=bv_d[:, :])
                nc.gpsimd.partition_broadcast(BVF[:, :], bv_row[:1, :])
                for dc in range(EC):
                    nc.sync.dma_start(out=WQ[:, dc, :], in_=wqT[:, dc, :])
                bq_raw = small.tile([128, EC], F32, tag="bq_raw")
                nc.sync.dma_start(out=bq_raw[:, :], in_=bq_d[:, :])
                nc.scalar.mul(BQ[:, :], bq_raw[:, :], inv)
                nc.sync.dma_start(out=MASK[:, :, :], in_=mask_d[:, :, :])
                bo_row = small.tile([1, D], F32, tag="bo_row")
                nc.sync.dma_start(out=bo_row[:, :], in_=bo_d[:, :])
                nc.gpsimd.partition_broadcast(BOF[:, :], bo_row[:1, :])

                # K-half: KTh[e, own-s]; sh-separated waves (sh0 first) so
                # the first K exchange fires early; dc outer for streaming
                def k_like_waves(WMAT, DST, bias_ap, scaled):
                    for sh in range(2):
                        for w in range(2):
                            ec0 = 4 * w
                            pss = [
                                pa_psum.tile(
                                    [128, 512], F32, tag="pa", name=f"pa_{sh}_{w}_{i}"
                                )
                                for i in range(4)
                            ]
                            for dc in range(EC):
                                for i in range(4):
                                    nc.tensor.matmul(
                                        pss[i][:, :],
                                        WMAT[:, dc, (ec0 + i) * 128 : (ec0 + i + 1) * 128],
                                        XQ[:, dc, sh * 512 : (sh + 1) * 512],
                                        start=(dc == 0),
                                        stop=(dc == EC - 1),
                                    )
                            for i in range(4):
                                ec = ec0 + i
                                if scaled:
                                    nc.vector.tensor_scalar(
                                        out=DST[:, ec, sh * 512 : (sh + 1) * 512],
                                        in0=pss[i][:, :],
                                        scalar1=inv,
                                        scalar2=bias_ap[:, ec : ec + 1],
                                        op0=mybir.AluOpType.mult,
                                        op1=mybir.AluOpType.add,
                                    )
                                else:
                                    nc.vector.tensor_scalar(
                                        out=DST[:, ec, sh * 512 : (sh + 1) * 512],
                                        in0=pss[i][:, :],
                                        scalar1=bias_ap[:, ec : ec + 1],
                                        scalar2=None,
                                        op0=mybir.AluOpType.add,
                                    )

                k_like_waves(WK, KTh, BK, scaled=False)
                # exchange K in halves; out-DMAs on the scalar HWDGE queue,
                # collectives + load-backs in-order on the gpsimd queue
                nc.scalar.dma_start(out=khalfA[:, :, :], in_=KTh[:, :, 0:512])
                nc.scalar.dma_start(out=khalfB[:, :, :], in_=KTh[:, :, 512:1024])
                nc.gpsimd.collective_compute(
                    "AllGather",
                    mybir.AluOpType.bypass,
                    replica_groups=GROUPS,
                    ins=[khalfA.opt()],
                    outs=[kgathA.opt()],
                )
                nc.gpsimd.collective_compute(
                    "AllGather",
                    mybir.AluOpType.bypass,
                    replica_groups=GROUPS,
                    ins=[khalfB.opt()],
                    outs=[kgathB.opt()],
                )

                # V-half: Vh[own-t, d]; slot-order waves (2 slots x 2 dh)
                Vv = V[:, :, :].rearrange("p (s b) d -> p s b d", b=2)
                for w in range(4):
                    s0 = 2 * w
                    pss = [
                        pa_psum.tile([128, 512], F32, tag="pa", name=f"pav{w}_{i}")
                        for i in range(4)
                    ]
                    for dc in range(EC):
                        for i, (sl, dh) in enumerate(
                            [(s0, 0), (s0, 1), (s0 + 1, 0), (s0 + 1, 1)]
                        ):
                            nc.tensor.matmul(
                                pss[i][:, :],
                                XQ[:, dc, sl * 128 : (sl + 1) * 128],
                                WV[:, dc, dh * 512 : (dh + 1) * 512],
                                start=(dc == 0),
                                stop=(dc == EC - 1),
                            )
                    for i, (sl, dh) in enumerate(
                        [(s0, 0), (s0, 1), (s0 + 1, 0), (s0 + 1, 1)]
                    ):
                        nc.vector.tensor_tensor(
                            out=Vh[:, sl, dh * 512 : (dh + 1) * 512],
                            in0=pss[i][:, :],
                            in1=BVF[:, dh * 512 : (dh + 1) * 512],
                            op=mybir.AluOpType.add,
                        )
                    if w == 1:
                        nc.scalar.dma_start(out=vhalfA[:, :, :], in_=Vh[:, 0:4, :])
                        nc.gpsimd.collective_compute(
                            "AllGather",
                            mybir.AluOpType.bypass,
                            replica_groups=GROUPS,
                            ins=[vhalfA.opt()],
                            outs=[vgathA.opt()],
                        )

                    if w == 3:
                        nc.scalar.dma_start(out=vhalfB[:, :, :], in_=Vh[:, 4:8, :])
                        nc.gpsimd.collective_compute(
                            "AllGather",
                            mybir.AluOpType.bypass,
                            replica_groups=GROUPS,
                            ins=[vhalfB.opt()],
                            outs=[vgathB.opt()],
                        )
                        # all load-backs AFTER every collective trigger:
                        # the CC engine is serial, so triggers must not wait
                        # behind a load-back on the in-order gpsimd queue
                        for r in range(2):
                            nc.scalar.dma_start(
                                out=KT[:, :, r, 0:512], in_=kgathA[r, :, :, :]
                            )
                        for r in range(2):
                            nc.scalar.dma_start(
                                out=KT[:, :, r, 512:1024], in_=kgathB[r, :, :, :]
                            )
                        for r in range(2):
                            nc.scalar.dma_start(
                                out=Vv[:, 0:4, r, :], in_=vgathA[r, :, :, :]
                            )
                        for r in range(2):
                            nc.scalar.dma_start(
                                out=Vv[:, 4:8, r, :], in_=vgathB[r, :, :, :]
                            )

                # QT (x 1/32, +bq/32), sh0 first so scores g0 unblock early
                k_like_waves(WQ, QT, BQ, scaled=True)

            # ---- phase B + C: attention + output projection ----
            with (
                tc.tile_pool(name="wo", bufs=1) as wo_pool,
                tc.tile_pool(name="att", bufs=5) as att_pool,
                tc.tile_pool(name="attT", bufs=2) as attT_pool,
                tc.tile_pool(name="ctx", bufs=1) as ctx_pool,
                tc.tile_pool(name="stat", bufs=1) as stat_pool,
                tc.tile_pool(name="sc_psum", bufs=3, space="PSUM") as sc_psum,
                tc.tile_pool(name="mm_psum", bufs=2, space="PSUM") as mm_psum,
                tc.tile_pool(name="outbuf", bufs=2) as out_pool,
            ):
                WO = wo_pool.tile([128, EC, D], BF16, tag="WO")
                for dc in range(EC):
                    nc.sync.dma_start(out=WO[:, dc, :], in_=woT[:, dc, :])
                CTXT = ctx_pool.tile([128, EC, 1024], BF16, tag="CTXT")
                NM = stat_pool.tile([128, 2], F32, tag="NM")
                LSUM = stat_pool.tile([128, 2], F32, tag="LS")
                LTOT = stat_pool.tile([128, 1], F32, tag="LT")

                def out_proj(slot):
                    OUTS = out_pool.tile([128, D], F32, tag="outs")
                    for eh in range(2):
                        ps = mm_psum.tile([128, 512], F32, tag="mm")
                        for dc in range(EC):
                            nc.tensor.matmul(
                                ps[:, :],
                                CTXT[:, dc, slot * 128 : (slot + 1) * 128],
                                WO[:, dc, eh * 512 : (eh + 1) * 512],
                                start=(dc == 0),
                                stop=(dc == EC - 1),
                            )
                        nc.vector.scalar_tensor_tensor(
                            out=OUTS[:, eh * 512 : (eh + 1) * 512],
                            in0=ps[:, :],
                            scalar=RL[:, slot : slot + 1],
                            in1=BOF[:, eh * 512 : (eh + 1) * 512],
                            op0=mybir.AluOpType.mult,
                            op1=mybir.AluOpType.add,
                        )
                    nc.sync.dma_start(
                        out=out_d[slot * 128 : (slot + 1) * 128, :], in_=OUTS[:, :]
                    )

                for g in range(2):
                    ATT_T = attT_pool.tile([128, NB, 512], BF16, tag="attT")
                    for j in range(4):
                        slot = g * 4 + j
                        T = _slot_T(slot)
                        nt = T // 128
                        ATT = att_pool.tile([128, S], BF16, tag="att")

                        nparts = (T + 1023) // 1024
                        parts = []
                        for p in range(nparts):
                            w = min(1024, T - p * 1024)
                            sc = sc_psum.tile([128, 1024], F32, tag="sc")
                            parts.append((sc, w))
                        # ec-outer: one LDWEIGHTS per ec covers the whole row
                        KTview = KT[:, :, :, :].rearrange(
                            "p e r (s c) -> p e s r c", c=128
                        )
                        for ec in range(EC):
                            for p, (sc, w) in enumerate(parts):
                                for c0 in range(0, w, 512):
                                    cw = min(512, w - c0)
                                    a0 = p * 1024 + c0
                                    s0, s1 = a0 // 256, (a0 + cw) // 256
                                    nc.tensor.matmul(
                                        sc[:, c0 : c0 + cw],
                                        QT[:, ec, slot * 128 : (slot + 1) * 128],
                                        KTview[:, ec, s0:s1, :, :],
                                        start=(ec == 0),
                                        stop=(ec == EC - 1),
                                    )
                        lsc, lw = parts[-1]
                        nc.vector.tensor_tensor(
                            out=lsc[:, lw - 256 : lw],
                            in0=lsc[:, lw - 256 : lw],
                            in1=MASK[:, slot, :],
                            op=mybir.AluOpType.add,
                        )
                        for p, (sc, w) in enumerate(parts):
                            nc.vector.reduce_max(
                                out=NM[:, p : p + 1],
                                in_=sc[:, :w],
                                axis=mybir.AxisListType.X,
                                negate=True,
                            )
                        if nparts == 2:
                            nc.vector.tensor_tensor(
                                out=NM[:, 0:1],
                                in0=NM[:, 0:1],
                                in1=NM[:, 1:2],
                                op=mybir.AluOpType.min,
                            )
                        for p, (sc, w) in enumerate(parts):
                            nc.scalar.activation(
                                ATT[:, p * 1024 : p * 1024 + w],
                                sc[:, :w],
                                mybir.ActivationFunctionType.Exp,
                                bias=NM[:, 0:1],
                                scale=1.0,
                                accum_out=LSUM[:, p : p + 1],
                            )
                        if nparts == 2:
                            nc.vector.tensor_tensor(
                                out=LTOT[:, 0:1],
                                in0=LSUM[:, 0:1],
                                in1=LSUM[:, 1:2],
                                op=mybir.AluOpType.add,
                            )
                            nc.vector.reciprocal(RL[:, slot : slot + 1], LTOT[:, 0:1])
                        else:
                            nc.vector.reciprocal(RL[:, slot : slot + 1], LSUM[:, 0:1])

                        nc.sync.dma_start_transpose(
                            ATT_T[:, 0:nt, j * 128 : (j + 1) * 128],
                            ATT[:, 0:T],
                        )

                    ntg = _slot_T(g * 4 + 3) // 128
                    for dc in range(EC):
                        ps = mm_psum.tile([128, 512], F32, tag="mm")
                        for tcn in range(ntg):
                            jmin = 0
                            for jj in range(4):
                                if 256 * (g * 4 + jj + 1) >= 128 * (tcn + 1):
                                    jmin = jj
                                    break
                            scol = jmin * 128
                            nc.tensor.matmul(
                                ps[:, scol:512],
                                V[:, tcn, dc * 128 : (dc + 1) * 128],
                                ATT_T[:, tcn, scol:512],
                                start=(tcn == 0),
                                stop=(tcn == ntg - 1),
                            )
                        nc.vector.tensor_copy(
                            CTXT[:, dc, g * 512 : (g + 1) * 512], ps[:, :]
                        )
                    for j in range(4):
                        out_proj(g * 4 + j)

    nc.compile()
    return nc


def _core_blocks(core):
    parity = 0 if core % 2 == 0 else 1  # even core (group rank 0) -> even blocks
    return [2 * s + parity for s in range(NSLOT)]


def _make_in_maps(x, Wq, bq, Wk, bk, Wv, bv, Wo, bo):
    bf = ml_dtypes.bfloat16

    def wt_layout(W):
        return np.ascontiguousarray(
            W.T.astype(bf).reshape(EC, 128, D).transpose(1, 0, 2)
        )

    wq_l, wk_l, wv_l, wo_l = (wt_layout(W) for W in (Wq, Wk, Wv, Wo))
    bq_l = np.ascontiguousarray(bq.reshape(EC, 128).T.astype(np.float32))
    bk_l = np.ascontiguousarray(bk.reshape(EC, 128).T.astype(np.float32))
    bv_l = np.ascontiguousarray(bv.reshape(1, D).astype(np.float32))
    bo_l = np.ascontiguousarray(bo.reshape(1, D).astype(np.float32))

    in_maps = []
    for core in range(8):
        b = core // 2
        blocks = _core_blocks(core)
        xb = np.asarray(x[b], dtype=np.float32)
        xq = np.concatenate([xb[bl * 128 : (bl + 1) * 128] for bl in blocks], axis=0)
        xqT_l = np.ascontiguousarray(
            xq.T.astype(bf).reshape(EC, 128, 1024).transpose(1, 0, 2)
        )
        mask = np.zeros((128, NSLOT, 256), np.float32)
        r = np.arange(128)[:, None]
        jj = np.arange(256)[None, :]
        for s_i, bl in enumerate(blocks):
            lim = bl * 128 + r
            t_idx = 256 * s_i + jj
            mask[:, s_i, :] = np.where(t_idx <= lim, 0.0, NEG)
        in_maps.append(
            {
                "xqT": xqT_l,
                "wqT": wq_l,
                "wkT": wk_l,
                "wvT": wv_l,
                "woT": wo_l,
                "bq": bq_l,
                "bk": bk_l,
                "bv": bv_l,
                "bo": bo_l,
                "mask": mask,
            }
        )
    return in_maps


def _run(inputs, trace=False):
    global _compiled
    if _compiled is None:
        _compiled = _build()
    nc = _compiled
    in_maps = _make_in_maps(**inputs)
    res = run_bass_kernel_spmd(nc, in_maps, core_ids=list(range(8)), trace=trace)
    out = np.zeros((B, S, D), np.float32)
    for core in range(8):
        b = core // 2
        o = res.results[core]["out"]
        for s_i, bl in enumerate(_core_blocks(core)):
            out[b, bl * 128 : (bl + 1) * 128, :] = o[s_i * 128 : (s_i + 1) * 128, :]
    return out, res


def kernel(**inputs):
    out, _ = _run(inputs, trace=False)
    return out
